# revision 15
# baseline (speedup 1.0000x reference)
"""LSTMCell on 8 Trainium2 NeuronCores, data-parallel over the batch.

Full inputs: x/h_t/c_t [65536,128] f32, 8 gate weight matrices [128,128],
4 biases [128]. Returns (h_new, c_new) as [65536,128] f32 each.

Design (v14): the v13 pipeline was ACT-bound: 5 transcendental columns
per batch element (4 gate sigmoids + tanh(c')) at the ACT engine's
fixed (N+352)/1.2ns => ~40us of ACT busy.  v14 moves tanh(c') OFF the
ACT engine into two custom fused DVE ops (injected per-NEFF via the
dve_ops extension point):
    LSTM_TANH_RECIP: r = 1/(c'^2+D) via BITWISE_NOT exponent-flip seed
        + one inline Newton step (fp32 internally, fp16 in/out)
    LSTM_TANH_ZMUL:  z = clamp((r*c2 + c1)*c', -1, 1) ~= tanh(c')
(minimax-fitted c1,c2,D: max |z - tanh| ~= 3.7e-3 over the fp16 path).
ACT now does ONLY the 4-bank sigmoid quad per group: 16 x 2000ns = 32us.

Engine balance per pair of 512-col groups (1024 batch cols):
  ACT  sigmoid quad [128,2048] x2           ~4.0us  <- pacer
  PE   16 fp16 matmuls (weights stationary) ~3.5us
  Pool u' = (sg-0.5)*si ; fc = sf*c         ~3.6us  (was idle)
  DVE  c' = 2u'+fc ; r ; z ; h' = z*so      ~3.8us
  DMA  in 0.75MB + out 0.5MB                ~3.6us
Gate order in quad/PSUM: [o | i | f | 2g] (g prescaled by 2 on host for
the tanh-via-sigmoid trick, fp16 operands as v13: bf16 rounding was the
dominant error term).

Fill/tail: x/h stream on the sync queue in consumption order; c stream
+ weights on the gpsimd queue (big c chunks issued between pool ops so
they can't steal DMA bandwidth from the x/h stream the PE waits on);
9 junk warmup matmuls keep the PE HAM activity window alive - fewer
leaves the PE clock-gated at half rate for the WHOLE kernel.  Last pair
runs its whole chain per-group on the DVE to shorten the kernel tail.
"""
import numpy as np
import ml_dtypes
from contextlib import ExitStack

import concourse.bass as bass
import concourse.tile as tile
from concourse import bacc, mybir
from concourse.bass_utils import run_bass_kernel_spmd

from concourse import dve_ops as _dop
from concourse.dve_spec import (
    Spec, Src0, Src1, C0, C1, C2, Bin, AluOp as DAlu, maxx, minn, sq,
    lower as _dve_lower, _has_src1,
)
from concourse.dve_uop import DveOpSpec

F32 = mybir.dt.float32
F16 = mybir.dt.float16
BF16 = mybir.dt.bfloat16
NPBF = ml_dtypes.bfloat16
AF = mybir.ActivationFunctionType
ALU = mybir.AluOpType

NCORES = 8
BC = 8192            # batch rows per core
GW = 512             # batch columns per group (one PSUM bank)
NG = BC // GW        # 16 groups
H = 128              # hidden size
# x/h chunks in groups (sync queue): small first for fast fill
ICHUNKS = [(0, 1), (1, 1), (2, 2), (4, 4), (8, 8)]
# c chunks in groups (gpsimd queue, pair-aligned)
CCHUNKS = [(0, 2), (2, 2), (4, 4), (8, 4), (12, 4)]
# output chunks (start group, n groups)
OCHUNKS = [(0, 8), (8, 4), (12, 2), (14, 2)]
# HAM un-throttle needs ~3.4us of SUSTAINED PE activity; at ~427ns cold
# issue per N=512 matmul that is >=8 warmups. Fewer warmups leave the PE
# at half clock for the WHOLE kernel (measured: 75us vs 58us).
NWARM = 9

# --- custom fused DVE ops (rational tanh) ---------------------------------
# recip seed/NR consts (same derivation as RECIPROCAL_APPROX_FAST's y1)
TR0, TR1 = -0.23549792, 2.0017324
# minimax fit of clamp(x*(TC1 + TC2/(x^2+TD))) ~= tanh(x) over the fp16
# path (joint fit including the approximate reciprocal): max err 3.7e-3
TC1, TC2, TD = 0.12762096, 2.40399202, 2.78807243


def _register_dve_op(name, spec, subdim=False):
    if name in _dop._SUB_OPCODE_FOR_NAME:
        return next(op for op in _dop.OPS if op.name == name)
    row = _dop._CUSTOM_DVE_ROW_BASE + len(_dop.OPS)
    assert row < 0x20, "custom DVE row overflow"
    shas = {}
    for ver in ("v3", "v4"):
        try:
            tmp = DveOpSpec(name=name, opcode=row,
                            uops=_dve_lower(spec, ver=ver),
                            rd1_en=_has_src1(spec))
            shas[ver] = tmp.sha(ver)
        except Exception:
            pass
    op = _dop.DveOp(name, spec, subdim, shas)
    _dop.OPS.append(op)
    _dop._SUB_OPCODE_FOR_NAME[name] = row
    _dop.CUSTOM_DVE_SPECS[name] = spec
    return op


def _ref_tanh_recip(in0, in1, c0, c1, c2):
    d = (in0.astype(np.float32) ** 2 + np.float32(c0)).astype(np.float32)
    nd = (~d.view(np.int32)).view(np.float32)
    y0 = nd * np.float32(c1)
    return y0 * (np.float32(c2) - d * y0)


def _ref_tanh_zmul(in0, in1, c0, c1, c2):
    t = ((in0.astype(np.float32) * np.float32(c0) + np.float32(c1))
         * in1.astype(np.float32))
    return np.maximum(np.minimum(t, np.float32(c2)), -np.float32(c2))


_d = sq(Src0) + C0
_nd = Bin(DAlu.BITWISE_NOT, _d, _d)
_y0 = _nd * C1
TANH_RECIP = _register_dve_op(
    "LSTM_TANH_RECIP", Spec(body=_y0 * (C2 - _d * _y0),
                            reference=_ref_tanh_recip))

_t = (Src0 * C0 + C1) * Src1
TANH_ZMUL = _register_dve_op(
    "LSTM_TANH_ZMUL", Spec(body=maxx(minn(_t, C2), -C2),
                           reference=_ref_tanh_zmul))

_CACHE = {}


def _build(has_bias: bool):
    nc = bacc.Bacc("TRN2", target_bir_lowering=False, debug=False)
    xt = nc.dram_tensor("xt", [H, BC], F16, kind="ExternalInput").ap()
    ht = nc.dram_tensor("ht", [H, BC], F16, kind="ExternalInput").ap()
    ct = nc.dram_tensor("ct", [H, BC], F16, kind="ExternalInput").ap()
    wxt = nc.dram_tensor("wxt", [H, 4 * H], F16, kind="ExternalInput").ap()
    wht = nc.dram_tensor("wht", [H, 4 * H], F16, kind="ExternalInput").ap()
    if has_bias:
        bias = nc.dram_tensor("bias", [H, 4], F32, kind="ExternalInput").ap()
    hnt = nc.dram_tensor("hnt", [H, BC], F16, kind="ExternalOutput").ap()
    cnt = nc.dram_tensor("cnt", [H, BC], F16, kind="ExternalOutput").ap()

    with tile.TileContext(nc) as tc:
        with ExitStack() as ctx:
            const = ctx.enter_context(tc.tile_pool(name="const", bufs=1))
            ina = ctx.enter_context(tc.tile_pool(name="ina", bufs=1))
            qp = ctx.enter_context(tc.tile_pool(name="qp", bufs=2, space="PSUM"))
            tp = ctx.enter_context(tc.tile_pool(name="tp", bufs=3))
            sp = ctx.enter_context(tc.tile_pool(name="sp", bufs=6))
            op = ctx.enter_context(tc.tile_pool(name="op", bufs=3))

            xts, hts = [], []
            for ci, (cs, cw) in enumerate(ICHUNKS):
                xts.append(ina.tile([H, cw * GW], F16, name=f"x{ci}"))
                hts.append(ina.tile([H, cw * GW], F16, name=f"h{ci}"))
            cts = [ina.tile([H, cw * GW], F16, name=f"c{ci}")
                   for ci, (cs, cw) in enumerate(CCHUNKS)]

            # gpsimd queue first: warmup/dummy memsets (so PE warmups are
            # not blocked behind DMA issues), then weights, then the two
            # small leading c chunks.  The big c chunks are issued later,
            # between pool ops, so their transfers can't crowd out the
            # x/h stream during the fill.  NOTE: never issue DMA from the
            # scalar queue - HWDGE on the Activation engine evicts its
            # ACT table (forces a ~1.3us reload).
            junk = const.tile([H, GW], F16)
            nc.gpsimd.memset(junk[:], 0.0)
            dummy = const.tile([H, 8], F32)
            nc.gpsimd.memset(dummy[:], 0.0)
            wx_sb = const.tile([H, 4 * H], F16)
            nc.gpsimd.dma_start(wx_sb[:], wxt)
            wh_sb = const.tile([H, 4 * H], F16)
            nc.gpsimd.dma_start(wh_sb[:], wht)
            if has_bias:
                b_sb = const.tile([H, 4], F32)
                nc.gpsimd.dma_start(b_sb[:], bias)

            def cstart(ci):
                cs, cw = CCHUNKS[ci]
                nc.gpsimd.dma_start(cts[ci][:], ct[:, cs * GW:(cs + cw) * GW])
            cstart(0)
            cstart(1)

            # sync queue: the x/h stream in consumption order
            for ci, (cs, cw) in enumerate(ICHUNKS):
                nc.sync.dma_start(xts[ci][:], xt[:, cs * GW:(cs + cw) * GW])
                nc.sync.dma_start(hts[ci][:], ht[:, cs * GW:(cs + cw) * GW])

            # ACT table preload (sigmoid) overlaps the DMA fill
            dummy2 = const.tile([H, 8], F32)
            nc.scalar.activation(dummy2[:], dummy[:], AF.Sigmoid)

            def in_slice(tiles, chunks, g, w):
                c0 = g * GW
                for ci, (cs, cw) in enumerate(chunks):
                    if c0 >= cs * GW and c0 + w <= (cs + cw) * GW:
                        return tiles[ci][:, c0 - cs * GW:c0 - cs * GW + w]
                raise AssertionError("slice straddles input chunks")

            # HAM warmup on the junk tile while DMAs stream
            warm = qp.tile([H, 2048], F32, name="warm", tag="quad")
            for _ in range(NWARM):
                nc.tensor.matmul(warm[:, 0:GW], junk[:, 0:H], junk[:],
                                 start=True, stop=True)

            NP = NG // 2  # pairs

            # pair -> (chunk_start_group, chunk_width, local_offset, is_last)
            pair_chunk = {}
            for cs, cw in OCHUNKS:
                for g in range(cs, cs + cw, 2):
                    pair_chunk[g // 2] = (cs, cw * GW, (g - cs) * GW,
                                          g + 2 == cs + cw)

            cn_hn = {}
            cn_buf = hn_buf = None
            sig2s = {}
            for P in range(NP):
                g0 = 2 * P
                cs, cw, lo, last = pair_chunk[P]
                if lo == 0:
                    cn_buf = op.tile([H, cw], F16, name=f"cn{g0}", tag="cn")
                    hn_buf = op.tile([H, cw], F16, name=f"hn{g0}", tag="hn")
                cn_hn[P] = (cn_buf, hn_buf)
                sig2 = sp.tile([H, 4096], BF16, name=f"s{P}", tag="sig")
                sig2s[P] = sig2
                lastP = P == NP - 1

                for gg in range(2):
                    g = g0 + gg
                    xs = in_slice(xts, ICHUNKS, g, GW)
                    hs = in_slice(hts, ICHUNKS, g, GW)
                    split = (lastP or P == 0) and not has_bias
                    quad = qp.tile([H, 2048], F32, name=f"q{g}", tag="quad")
                    so = sig2[:, gg * 2048:(gg + 1) * 2048]
                    for k in ([1, 2, 3, 0] if split else range(4)):
                        nc.tensor.matmul(quad[:, k * GW:(k + 1) * GW],
                                         wx_sb[:, k * H:(k + 1) * H], xs,
                                         start=True, stop=False)
                        nc.tensor.matmul(quad[:, k * GW:(k + 1) * GW],
                                         wh_sb[:, k * H:(k + 1) * H], hs,
                                         start=False, stop=True)
                    if has_bias:
                        for k in range(4):
                            nc.scalar.activation(
                                so[:, k * GW:(k + 1) * GW],
                                quad[:, k * GW:(k + 1) * GW],
                                AF.Sigmoid, bias=b_sb[:, k:k + 1])
                    elif split:
                        # i/f/2g banks first: unblocks the pool/DVE chain;
                        # the o bank (only needed by h') trails
                        nc.scalar.activation(so[:, GW:], quad[:, GW:],
                                             AF.Sigmoid)
                        nc.scalar.activation(so[:, 0:GW], quad[:, 0:GW],
                                             AF.Sigmoid)
                    else:
                        nc.scalar.activation(so, quad[:], AF.Sigmoid)

                def sl(bank, gg=None):
                    s = sig2[:].rearrange("p (t x) -> p t x", t=2)
                    s = s[:, :, bank * GW:(bank + 1) * GW]
                    if gg is not None:
                        s = s[:, gg:gg + 1, :]
                    return s

                def r3(ap2d, t=2):
                    return ap2d.rearrange("p (t x) -> p t x", t=t)

                if lastP:
                    # tail: whole chain per-group on the DVE (pool's slower
                    # ops would sit on the critical path) + per-group DMAs
                    for gg in range(2):
                        g = g0 + gg
                        lg = lo + gg * GW
                        cps = cn_buf[:, lg:lg + GW]
                        up = tp.tile([H, GW], F16, name=f"up{g}", tag="up")
                        nc.vector.scalar_tensor_tensor(
                            up[:], sl(3, gg)[:, 0, :], 0.5, sl(1, gg)[:, 0, :],
                            ALU.subtract, ALU.mult)
                        fc = tp.tile([H, GW], F16, name=f"fc{g}", tag="fc")
                        nc.vector.tensor_mul(
                            fc[:], sl(2, gg)[:, 0, :],
                            in_slice(cts, CCHUNKS, g, GW))
                        nc.vector.scalar_tensor_tensor(
                            cps, up[:], 2.0, fc[:], ALU.mult, ALU.add)
                        nc.sync.dma_start(
                            cnt[:, (cs + gg * (cw // GW - 1)) * GW:
                                (cs + gg * (cw // GW - 1)) * GW + GW], cps)
                        rt = tp.tile([H, GW], F16, name=f"r{g}", tag="rt")
                        nc.vector._custom_dve(TANH_RECIP, out=rt[:], in0=cps,
                                              s0=TD, s1=TR0, imm2=TR1)
                        zt = tp.tile([H, GW], F16, name=f"z{g}", tag="zt")
                        nc.vector._custom_dve(TANH_ZMUL, out=zt[:], in0=rt[:],
                                              in1=cps, s0=TC2, s1=TC1,
                                              imm2=1.0)
                        nc.vector.tensor_mul(hn_buf[:, lg:lg + GW],
                                             sl(0, gg)[:, 0, :], zt[:])
                        nc.sync.dma_start(
                            hnt[:, (cs + gg * (cw // GW - 1)) * GW:
                                (cs + gg * (cw // GW - 1)) * GW + GW],
                            hn_buf[:, lg:lg + GW])
                    continue

                W2 = 2 * GW
                cpr = cn_buf[:, lo:lo + W2]
                # pool: fc = sf * c (plain tensor_tensor - the only op kind
                # walrus accepts on Pool); u' on the DVE
                fc = tp.tile([H, W2], F16, name=f"fc{P}", tag="fc")
                nc.gpsimd.tensor_mul(r3(fc[:]), sl(2),
                                     r3(in_slice(cts, CCHUNKS, g0, W2)))
                up = tp.tile([H, W2], F16, name=f"up{P}", tag="up")
                nc.vector.scalar_tensor_tensor(
                    r3(up[:]), sl(3), 0.5, sl(1), ALU.subtract, ALU.mult)
                # big c chunks issued from the pool queue, paced by compute
                if P == 0:
                    cstart(2)
                    cstart(3)
                elif P == 1:
                    cstart(4)
                # DVE: c' = 2u' + fc ; r ; z=tanh(c') ; h' = z * so
                nc.vector.scalar_tensor_tensor(
                    cpr, up[:], 2.0, fc[:], ALU.mult, ALU.add)
                if last:
                    nc.sync.dma_start(cnt[:, cs * GW:cs * GW + cw], cn_buf[:])
                rt = tp.tile([H, W2], F16, name=f"r{P}", tag="rt")
                nc.vector._custom_dve(TANH_RECIP, out=rt[:], in0=cpr,
                                      s0=TD, s1=TR0, imm2=TR1)
                zt = tp.tile([H, W2], F16, name=f"z{P}", tag="zt")
                nc.vector._custom_dve(TANH_ZMUL, out=zt[:], in0=rt[:],
                                      in1=cpr, s0=TC2, s1=TC1, imm2=1.0)
                nc.vector.tensor_mul(r3(hn_buf[:, lo:lo + W2]), sl(0),
                                     r3(zt[:]))
                if last:
                    nc.sync.dma_start(hnt[:, cs * GW:cs * GW + cw], hn_buf[:])
    nc.compile()
    return nc


def _run(inputs, trace=False, tmpdir=None):
    x = np.asarray(inputs["x"], dtype=np.float32)
    h = np.asarray(inputs["h_t"], dtype=np.float32)
    c = np.asarray(inputs["c_t"], dtype=np.float32)
    # gate order [o, i, f, g]; W_g/b_g scaled by 2 for the tanh-via-sigmoid
    wx = np.concatenate([inputs["W_io"], inputs["W_ii"], inputs["W_if"],
                         2.0 * np.asarray(inputs["W_ig"])], axis=0)
    wh = np.concatenate([inputs["W_ho"], inputs["W_hi"], inputs["W_hf"],
                         2.0 * np.asarray(inputs["W_hg"])], axis=0)
    b = np.concatenate([inputs["b_o"], inputs["b_i"], inputs["b_f"],
                        2.0 * np.asarray(inputs["b_g"])], axis=0)
    wxt = np.ascontiguousarray(wx.T).astype(np.float16)
    wht = np.ascontiguousarray(wh.T).astype(np.float16)
    has_bias = bool(np.any(b))

    key = has_bias
    if key not in _CACHE:
        _CACHE[key] = _build(has_bias)
    nc = _CACHE[key]

    x16 = x.astype(np.float16)
    h16 = h.astype(np.float16)
    c16 = c.astype(np.float16)
    in_maps = []
    for i in range(NCORES):
        s = slice(i * BC, (i + 1) * BC)
        m = {
            "xt": np.ascontiguousarray(x16[s].T),
            "ht": np.ascontiguousarray(h16[s].T),
            "ct": np.ascontiguousarray(c16[s].T),
            "wxt": wxt,
            "wht": wht,
        }
        if has_bias:
            m["bias"] = np.ascontiguousarray(
                b.reshape(4, H).T.astype(np.float32))
        in_maps.append(m)

    res = run_bass_kernel_spmd(nc, in_maps, core_ids=list(range(NCORES)),
                               trace=trace, tmpdir=tmpdir)
    h_new = np.empty((NCORES * BC, H), dtype=np.float32)
    c_new = np.empty((NCORES * BC, H), dtype=np.float32)
    for i, r in enumerate(res.results):
        s = slice(i * BC, (i + 1) * BC)
        h_new[s] = r["hnt"].T
        c_new[s] = r["cnt"].T
    return h_new, c_new, res


def kernel(**inputs):
    h_new, c_new, _ = _run(inputs, trace=False)
    return h_new, c_new


# revision 19
# speedup vs baseline: 1.0008x; 1.0008x over previous
"""LSTMCell on 8 Trainium2 NeuronCores, data-parallel over the batch.

Full inputs: x/h_t/c_t [65536,128] f32, 8 gate weight matrices [128,128],
4 biases [128]. Returns (h_new, c_new) as [65536,128] f32 each.

Design (v14): the v13 pipeline was ACT-bound: 5 transcendental columns
per batch element (4 gate sigmoids + tanh(c')) at the ACT engine's
fixed (N+352)/1.2ns => ~40us of ACT busy.  v14 moves tanh(c') OFF the
ACT engine into two custom fused DVE ops (injected per-NEFF via the
dve_ops extension point):
    LSTM_TANH_RECIP: r = 1/(c'^2+D) via BITWISE_NOT exponent-flip seed
        + one inline Newton step (fp32 internally, fp16 in/out)
    LSTM_TANH_ZMUL:  z = clamp((r*c2 + c1)*c', -1, 1) ~= tanh(c')
(minimax-fitted c1,c2,D: max |z - tanh| ~= 3.7e-3 over the fp16 path).
ACT now does ONLY the 4-bank sigmoid quad per group: 16 x 2000ns = 32us.

Engine balance per pair of 512-col groups (1024 batch cols):
  ACT  sigmoid quad [128,2048] x2           ~4.0us  <- pacer
  PE   16 fp16 matmuls (weights stationary) ~3.5us
  Pool u' = (sg-0.5)*si ; fc = sf*c         ~3.6us  (was idle)
  DVE  c' = 2u'+fc ; r ; z ; h' = z*so      ~3.8us
  DMA  in 0.75MB + out 0.5MB                ~3.6us
Gate order in quad/PSUM: [o | i | f | 2g] (g prescaled by 2 on host for
the tanh-via-sigmoid trick, fp16 operands as v13: bf16 rounding was the
dominant error term).

Fill/tail: x/h stream on the sync queue in consumption order; c stream
+ weights on the gpsimd queue (big c chunks issued between pool ops so
they can't steal DMA bandwidth from the x/h stream the PE waits on);
9 junk warmup matmuls keep the PE HAM activity window alive - fewer
leaves the PE clock-gated at half rate for the WHOLE kernel.  Last pair
runs its whole chain per-group on the DVE to shorten the kernel tail.
"""
import numpy as np
import ml_dtypes
from contextlib import ExitStack

import concourse.bass as bass
import concourse.tile as tile
from concourse import bacc, mybir
from concourse.bass_utils import run_bass_kernel_spmd

from concourse import dve_ops as _dop
from concourse.dve_spec import (
    Spec, Src0, Src1, C0, C1, C2, Bin, AluOp as DAlu, maxx, minn, sq,
    lower as _dve_lower, _has_src1,
)
from concourse.dve_uop import DveOpSpec

F32 = mybir.dt.float32
F16 = mybir.dt.float16
BF16 = mybir.dt.bfloat16
NPBF = ml_dtypes.bfloat16
AF = mybir.ActivationFunctionType
ALU = mybir.AluOpType

NCORES = 8
BC = 8192            # batch rows per core
GW = 512             # batch columns per group (one PSUM bank)
NG = BC // GW        # 16 groups
H = 128              # hidden size
# x/h chunks in groups (sync queue): small first for fast fill
ICHUNKS = [(0, 1), (1, 1), (2, 2), (4, 4), (8, 8)]
# c chunks in groups (gpsimd queue, pair-aligned)
CCHUNKS = [(0, 2), (2, 2), (4, 4), (8, 4), (12, 4)]
# output chunks (start group, n groups)
OCHUNKS = [(0, 8), (8, 4), (12, 2), (14, 2)]
# HAM un-throttle needs ~3.4us of SUSTAINED PE activity; at ~427ns cold
# issue per N=512 matmul that is >=8 warmups. Fewer warmups leave the PE
# at half clock for the WHOLE kernel (measured: 75us vs 58us).
NWARM = 9

# --- custom fused DVE ops (rational tanh) ---------------------------------
# recip seed/NR consts (same derivation as RECIPROCAL_APPROX_FAST's y1)
TR0, TR1 = -0.23549792, 2.0017324
# minimax fit of clamp(x*(TC1 + TC2/(x^2+TD))) ~= tanh(x) over the fp16
# path (joint fit including the approximate reciprocal): max err 3.7e-3
TC1, TC2, TD = 0.12762096, 2.40399202, 2.78807243


def _register_dve_op(name, spec, subdim=False):
    if name in _dop._SUB_OPCODE_FOR_NAME:
        return next(op for op in _dop.OPS if op.name == name)
    row = _dop._CUSTOM_DVE_ROW_BASE + len(_dop.OPS)
    assert row < 0x20, "custom DVE row overflow"
    shas = {}
    for ver in ("v3", "v4"):
        try:
            tmp = DveOpSpec(name=name, opcode=row,
                            uops=_dve_lower(spec, ver=ver),
                            rd1_en=_has_src1(spec))
            shas[ver] = tmp.sha(ver)
        except Exception:
            pass
    op = _dop.DveOp(name, spec, subdim, shas)
    _dop.OPS.append(op)
    _dop._SUB_OPCODE_FOR_NAME[name] = row
    _dop.CUSTOM_DVE_SPECS[name] = spec
    return op


def _ref_tanh_recip(in0, in1, c0, c1, c2):
    d = (in0.astype(np.float32) ** 2 + np.float32(c0)).astype(np.float32)
    nd = (~d.view(np.int32)).view(np.float32)
    y0 = nd * np.float32(c1)
    return y0 * (np.float32(c2) - d * y0)


def _ref_tanh_zmul(in0, in1, c0, c1, c2):
    t = ((in0.astype(np.float32) * np.float32(c0) + np.float32(c1))
         * in1.astype(np.float32))
    return np.maximum(np.minimum(t, np.float32(c2)), -np.float32(c2))


_d = sq(Src0) + C0
_nd = Bin(DAlu.BITWISE_NOT, _d, _d)
_y0 = _nd * C1
TANH_RECIP = _register_dve_op(
    "LSTM_TANH_RECIP", Spec(body=_y0 * (C2 - _d * _y0),
                            reference=_ref_tanh_recip))

_t = (Src0 * C0 + C1) * Src1
TANH_ZMUL = _register_dve_op(
    "LSTM_TANH_ZMUL", Spec(body=maxx(minn(_t, C2), -C2),
                           reference=_ref_tanh_zmul))

# clamp-free (Src0*c0 + c1)*Src1 - no imm2, so 3D (2-free-dim) operands
# are allowed (STT struct). Used for ig = (2*sg - 1)*si.
AFFINE_MUL = _register_dve_op(
    "LSTM_AFFINE_MUL",
    Spec(body=_t, reference=lambda in0, in1, c0, c1, c2:
         (in0.astype(np.float32) * np.float32(c0) + np.float32(c1))
         * in1.astype(np.float32)))

_CACHE = {}


def _build(has_bias: bool):
    nc = bacc.Bacc("TRN2", target_bir_lowering=False, debug=False)
    xt = nc.dram_tensor("xt", [H, BC], F16, kind="ExternalInput").ap()
    ht = nc.dram_tensor("ht", [H, BC], F16, kind="ExternalInput").ap()
    ct = nc.dram_tensor("ct", [H, BC], F16, kind="ExternalInput").ap()
    wxt = nc.dram_tensor("wxt", [H, 4 * H], F16, kind="ExternalInput").ap()
    wht = nc.dram_tensor("wht", [H, 4 * H], F16, kind="ExternalInput").ap()
    if has_bias:
        bias = nc.dram_tensor("bias", [H, 4], F32, kind="ExternalInput").ap()
    hnt = nc.dram_tensor("hnt", [H, BC], F16, kind="ExternalOutput").ap()
    cnt = nc.dram_tensor("cnt", [H, BC], F16, kind="ExternalOutput").ap()

    with tile.TileContext(nc) as tc:
        with ExitStack() as ctx:
            const = ctx.enter_context(tc.tile_pool(name="const", bufs=1))
            ina = ctx.enter_context(tc.tile_pool(name="ina", bufs=1))
            qp = ctx.enter_context(tc.tile_pool(name="qp", bufs=2, space="PSUM"))
            tp = ctx.enter_context(tc.tile_pool(name="tp", bufs=3))
            sp = ctx.enter_context(tc.tile_pool(name="sp", bufs=6))
            op = ctx.enter_context(tc.tile_pool(name="op", bufs=3))

            xts, hts = [], []
            for ci, (cs, cw) in enumerate(ICHUNKS):
                xts.append(ina.tile([H, cw * GW], F16, name=f"x{ci}"))
                hts.append(ina.tile([H, cw * GW], F16, name=f"h{ci}"))
            cts = [ina.tile([H, cw * GW], F16, name=f"c{ci}")
                   for ci, (cs, cw) in enumerate(CCHUNKS)]

            # gpsimd queue first: warmup/dummy memsets (so PE warmups are
            # not blocked behind DMA issues), then weights, then the two
            # small leading c chunks.  The big c chunks are issued later,
            # between pool ops, so their transfers can't crowd out the
            # x/h stream during the fill.  NOTE: never issue DMA from the
            # scalar queue - HWDGE on the Activation engine evicts its
            # ACT table (forces a ~1.3us reload).
            junk = const.tile([H, GW], F16)
            nc.gpsimd.memset(junk[:], 0.0)
            dummy = const.tile([H, 8], F32)
            nc.gpsimd.memset(dummy[:], 0.0)
            wx_sb = const.tile([H, 4 * H], F16)
            nc.gpsimd.dma_start(wx_sb[:], wxt)
            wh_sb = const.tile([H, 4 * H], F16)
            nc.gpsimd.dma_start(wh_sb[:], wht)
            if has_bias:
                b_sb = const.tile([H, 4], F32)
                nc.gpsimd.dma_start(b_sb[:], bias)

            def cstart(ci):
                cs, cw = CCHUNKS[ci]
                nc.gpsimd.dma_start(cts[ci][:], ct[:, cs * GW:(cs + cw) * GW])
            cstart(0)
            cstart(1)

            # sync queue: the x/h stream in consumption order
            for ci, (cs, cw) in enumerate(ICHUNKS):
                nc.sync.dma_start(xts[ci][:], xt[:, cs * GW:(cs + cw) * GW])
                nc.sync.dma_start(hts[ci][:], ht[:, cs * GW:(cs + cw) * GW])

            # ACT table preload (sigmoid) overlaps the DMA fill
            dummy2 = const.tile([H, 8], F32)
            nc.scalar.activation(dummy2[:], dummy[:], AF.Sigmoid)

            def in_slice(tiles, chunks, g, w):
                c0 = g * GW
                for ci, (cs, cw) in enumerate(chunks):
                    if c0 >= cs * GW and c0 + w <= (cs + cw) * GW:
                        return tiles[ci][:, c0 - cs * GW:c0 - cs * GW + w]
                raise AssertionError("slice straddles input chunks")

            # HAM warmup on the junk tile while DMAs stream
            warm = qp.tile([H, 2048], F32, name="warm", tag="quad")
            for _ in range(NWARM):
                nc.tensor.matmul(warm[:, 0:GW], junk[:, 0:H], junk[:],
                                 start=True, stop=True)

            NP = NG // 2  # pairs

            # pair -> (chunk_start_group, chunk_width, local_offset, is_last)
            pair_chunk = {}
            for cs, cw in OCHUNKS:
                for g in range(cs, cs + cw, 2):
                    pair_chunk[g // 2] = (cs, cw * GW, (g - cs) * GW,
                                          g + 2 == cs + cw)

            cn_hn = {}
            cn_buf = hn_buf = None
            sig2s = {}
            for P in range(NP):
                g0 = 2 * P
                cs, cw, lo, last = pair_chunk[P]
                if lo == 0:
                    cn_buf = op.tile([H, cw], F16, name=f"cn{g0}", tag="cn")
                    hn_buf = op.tile([H, cw], F16, name=f"hn{g0}", tag="hn")
                cn_hn[P] = (cn_buf, hn_buf)
                sig2 = sp.tile([H, 4096], BF16, name=f"s{P}", tag="sig")
                sig2s[P] = sig2
                lastP = P == NP - 1

                for gg in range(2):
                    g = g0 + gg
                    xs = in_slice(xts, ICHUNKS, g, GW)
                    hs = in_slice(hts, ICHUNKS, g, GW)
                    split = (lastP or P == 0) and not has_bias
                    quad = qp.tile([H, 2048], F32, name=f"q{g}", tag="quad")
                    so = sig2[:, gg * 2048:(gg + 1) * 2048]
                    for k in ([1, 2, 3, 0] if split else range(4)):
                        nc.tensor.matmul(quad[:, k * GW:(k + 1) * GW],
                                         wx_sb[:, k * H:(k + 1) * H], xs,
                                         start=True, stop=False)
                        nc.tensor.matmul(quad[:, k * GW:(k + 1) * GW],
                                         wh_sb[:, k * H:(k + 1) * H], hs,
                                         start=False, stop=True)
                    if has_bias:
                        for k in range(4):
                            nc.scalar.activation(
                                so[:, k * GW:(k + 1) * GW],
                                quad[:, k * GW:(k + 1) * GW],
                                AF.Sigmoid, bias=b_sb[:, k:k + 1])
                    elif split:
                        # i/f/2g banks first: unblocks the pool/DVE chain;
                        # the o bank (only needed by h') trails
                        nc.scalar.activation(so[:, GW:], quad[:, GW:],
                                             AF.Sigmoid)
                        nc.scalar.activation(so[:, 0:GW], quad[:, 0:GW],
                                             AF.Sigmoid)
                    else:
                        nc.scalar.activation(so, quad[:], AF.Sigmoid)

                def sl(bank, gg=None):
                    s = sig2[:].rearrange("p (t x) -> p t x", t=2)
                    s = s[:, :, bank * GW:(bank + 1) * GW]
                    if gg is not None:
                        s = s[:, gg:gg + 1, :]
                    return s

                def r3(ap2d, t=2):
                    return ap2d.rearrange("p (t x) -> p t x", t=t)

                if lastP:
                    # tail: whole chain per-group on the DVE (pool's slower
                    # ops would sit on the critical path) + per-group DMAs
                    for gg in range(2):
                        g = g0 + gg
                        lg = lo + gg * GW
                        cps = cn_buf[:, lg:lg + GW]
                        ig = tp.tile([H, GW], F16, name=f"ig{g}", tag="ig")
                        nc.vector._custom_dve(
                            AFFINE_MUL, out=ig[:], in0=sl(3, gg)[:, 0, :],
                            in1=sl(1, gg)[:, 0, :], s0=2.0, s1=-1.0)
                        fc = tp.tile([H, GW], F16, name=f"fc{g}", tag="fc")
                        nc.vector.tensor_mul(
                            fc[:], sl(2, gg)[:, 0, :],
                            in_slice(cts, CCHUNKS, g, GW))
                        nc.vector.tensor_add(cps, ig[:], fc[:])
                        nc.sync.dma_start(
                            cnt[:, (cs + gg * (cw // GW - 1)) * GW:
                                (cs + gg * (cw // GW - 1)) * GW + GW], cps)
                        rt = tp.tile([H, GW], F16, name=f"r{g}", tag="rt")
                        nc.vector._custom_dve(TANH_RECIP, out=rt[:], in0=cps,
                                              s0=TD, s1=TR0, imm2=TR1)
                        zt = tp.tile([H, GW], F16, name=f"z{g}", tag="zt")
                        nc.vector._custom_dve(TANH_ZMUL, out=zt[:], in0=rt[:],
                                              in1=cps, s0=TC2, s1=TC1,
                                              imm2=1.0)
                        nc.vector.tensor_mul(hn_buf[:, lg:lg + GW],
                                             sl(0, gg)[:, 0, :], zt[:])
                        nc.sync.dma_start(
                            hnt[:, (cs + gg * (cw // GW - 1)) * GW:
                                (cs + gg * (cw // GW - 1)) * GW + GW],
                            hn_buf[:, lg:lg + GW])
                    continue

                W2 = 2 * GW
                cpr = cn_buf[:, lo:lo + W2]
                # pool: fc = sf * c (plain tensor_tensor - the only op kind
                # walrus accepts on Pool); later also h' = so * z
                fc = tp.tile([H, W2], F16, name=f"fc{P}", tag="fc")
                nc.gpsimd.tensor_mul(r3(fc[:]), sl(2),
                                     r3(in_slice(cts, CCHUNKS, g0, W2)))
                # DVE: ig = (2*sg - 1) * si  (TANH_ZMUL reused: its +-1
                # clamp is a no-op since |ig| < 1 by construction)
                ig = tp.tile([H, W2], F16, name=f"ig{P}", tag="ig")
                nc.vector._custom_dve(AFFINE_MUL, out=r3(ig[:]), in0=sl(3),
                                      in1=sl(1), s0=2.0, s1=-1.0)
                # big c chunks issued from the pool queue, paced by compute
                if P == 0:
                    cstart(2)
                    cstart(3)
                elif P == 1:
                    cstart(4)
                # DVE: c' = ig + fc ; r ; z=tanh(c').  pool: h' = so * z
                nc.vector.tensor_add(cpr, ig[:], fc[:])
                if last:
                    nc.sync.dma_start(cnt[:, cs * GW:cs * GW + cw], cn_buf[:])
                rt = tp.tile([H, W2], F16, name=f"r{P}", tag="rt")
                nc.vector._custom_dve(TANH_RECIP, out=rt[:], in0=cpr,
                                      s0=TD, s1=TR0, imm2=TR1)
                zt = tp.tile([H, W2], F16, name=f"z{P}", tag="zt")
                nc.vector._custom_dve(TANH_ZMUL, out=zt[:], in0=rt[:],
                                      in1=cpr, s0=TC2, s1=TC1, imm2=1.0)
                nc.gpsimd.tensor_mul(r3(hn_buf[:, lo:lo + W2]), sl(0),
                                     r3(zt[:]))
                if last:
                    nc.sync.dma_start(hnt[:, cs * GW:cs * GW + cw], hn_buf[:])
    nc.compile()
    return nc


def _run(inputs, trace=False, tmpdir=None):
    x = np.asarray(inputs["x"], dtype=np.float32)
    h = np.asarray(inputs["h_t"], dtype=np.float32)
    c = np.asarray(inputs["c_t"], dtype=np.float32)
    # gate order [o, i, f, g]; W_g/b_g scaled by 2 for the tanh-via-sigmoid
    wx = np.concatenate([inputs["W_io"], inputs["W_ii"], inputs["W_if"],
                         2.0 * np.asarray(inputs["W_ig"])], axis=0)
    wh = np.concatenate([inputs["W_ho"], inputs["W_hi"], inputs["W_hf"],
                         2.0 * np.asarray(inputs["W_hg"])], axis=0)
    b = np.concatenate([inputs["b_o"], inputs["b_i"], inputs["b_f"],
                        2.0 * np.asarray(inputs["b_g"])], axis=0)
    wxt = np.ascontiguousarray(wx.T).astype(np.float16)
    wht = np.ascontiguousarray(wh.T).astype(np.float16)
    has_bias = bool(np.any(b))

    key = has_bias
    if key not in _CACHE:
        _CACHE[key] = _build(has_bias)
    nc = _CACHE[key]

    x16 = x.astype(np.float16)
    h16 = h.astype(np.float16)
    c16 = c.astype(np.float16)
    in_maps = []
    for i in range(NCORES):
        s = slice(i * BC, (i + 1) * BC)
        m = {
            "xt": np.ascontiguousarray(x16[s].T),
            "ht": np.ascontiguousarray(h16[s].T),
            "ct": np.ascontiguousarray(c16[s].T),
            "wxt": wxt,
            "wht": wht,
        }
        if has_bias:
            m["bias"] = np.ascontiguousarray(
                b.reshape(4, H).T.astype(np.float32))
        in_maps.append(m)

    res = run_bass_kernel_spmd(nc, in_maps, core_ids=list(range(NCORES)),
                               trace=trace, tmpdir=tmpdir)
    h_new = np.empty((NCORES * BC, H), dtype=np.float32)
    c_new = np.empty((NCORES * BC, H), dtype=np.float32)
    for i, r in enumerate(res.results):
        s = slice(i * BC, (i + 1) * BC)
        h_new[s] = r["hnt"].T
        c_new[s] = r["cnt"].T
    return h_new, c_new, res


def kernel(**inputs):
    h_new, c_new, _ = _run(inputs, trace=False)
    return h_new, c_new
